# revision 1
# baseline (speedup 1.0000x reference)
"""GCN (2x GCNConv + graclus-style max-pool head) on 8 Trainium2 NeuronCores.

Strategy (graph partitioning per the sharding hint):
  - Nodes are sharded contiguously across 8 cores (12500 each, padded to
    12544 = 98 tiles of 128).  Edges are partitioned by destination node.
  - deg / dinv = 1/sqrt(deg) are computed fully locally (all edges with a
    given dst live on its owner core).
  - Per layer: each core computes dinv * (x_shard @ W) locally, then an
    8-rank AllGather replicates the full [100352, 64] feature table.
  - Edge pass: per 128-edge chunk, dma_gather pulls table[src] rows into
    SBUF, a selection matrix sel[e, n] = (iota[n] == dst_local[e]) * w[e]
    is built with one fused tensor_scalar op, and the TensorEngine
    accumulates sel.T @ gathered into the per-tile PSUM (segment-sum).
    Self loops are included as ordinary edges with w = 1.
  - Pooling head: out[b] = max(h2[2c], h2[2c+1]) for the first cluster c of
    graph b; those 512 rows are fetched with one small dma_gather and
    reduced with a single elementwise max.
"""

import os
import sys

sys.path.insert(0, "/opt/trn_rl_repo")

import numpy as np

N = 100000
E = 1600000
B = 256
IN_DIM = 128
OUT_DIM = 64
NCORES = 8
NS = N // NCORES          # 12500 real nodes per core
NT = (NS + 127) // 128    # 98 tiles per core
NSP = NT * 128            # 12544 padded nodes per core
GT = 7                    # tiles per gather group
NG = NT // GT             # 14 groups
NSH = 4                   # src table shards (int16 gather index limit)
SHR = 2 * NSP             # 25088 rows per shard
TOTR = NCORES * NSP       # 100352 table rows
P = 128
D = OUT_DIM


def _prepare(inputs):
    x = np.asarray(inputs["x"], dtype=np.float32)
    edge_index = np.asarray(inputs["edge_index"], dtype=np.int64)
    edge_weight = np.asarray(inputs["edge_weight"], dtype=np.float32)
    batch = np.asarray(inputs["batch"], dtype=np.int64)
    W1 = np.asarray(inputs["W1"], dtype=np.float32)
    b1 = np.asarray(inputs["b1"], dtype=np.float32)
    W2 = np.asarray(inputs["W2"], dtype=np.float32)
    b2 = np.asarray(inputs["b2"], dtype=np.float32)

    src0 = edge_index[0]
    dst0 = edge_index[1]
    loop = np.arange(N, dtype=np.int64)
    src = np.concatenate([src0, loop])
    dst = np.concatenate([dst0, loop])
    w = np.concatenate([edge_weight, np.ones(N, np.float32)])

    core = dst // NS
    lt = dst - core * NS            # local node id 0..12499
    t = lt // P                     # tile 0..97
    dl = (lt - t * P).astype(np.float32)
    r = (src // NS) * NSP + (src % NS)   # padded table row
    s = r // SHR                    # src shard 0..3
    li = (r - s * SHR).astype(np.int16)

    # per (core, tile, shard) counts -> global chunk capacities K[t, s]
    key = ((core * NT + t) * NSH + s).astype(np.int64)
    cnt = np.bincount(key, minlength=NCORES * NT * NSH).reshape(NCORES, NT, NSH)
    K = ((cnt.max(axis=0) + P - 1) // P).astype(np.int64)   # [NT, NSH]

    # group-local layout: for group g: for s: for t in g: K[t,s] chunks
    cb0 = np.zeros((NG, NSH), np.int64)       # group-local chunk base of (g, s)
    toff = np.zeros((NT, NSH), np.int64)      # chunk offset of tile within (g, s)
    NIgs = np.zeros((NG, NSH), np.int64)      # idxs per gather instruction
    Cg = np.zeros(NG, np.int64)               # chunks per group
    for g in range(NG):
        tl = range(g * GT, (g + 1) * GT)
        off = 0
        for sh in range(NSH):
            cb0[g, sh] = off
            o2 = 0
            for tt in tl:
                toff[tt, sh] = o2
                o2 += K[tt, sh]
            NIgs[g, sh] = P * o2
            off += o2
        Cg[g] = off
    gbase = np.concatenate([[0], np.cumsum(Cg)])   # group chunk base, [NG+1]
    TC = int(gbase[-1])                            # total chunks per layer
    icb = np.zeros((NG, NSH), np.int64)            # idx col base per instruction
    run = 0
    for g in range(NG):
        for sh in range(NSH):
            icb[g, sh] = run
            run += NIgs[g, sh] // 16
    ICOLS = int(run)

    # global slot base of each (t, s) cell
    gidx_of_t = np.arange(NT) // GT
    cellbase = P * (gbase[gidx_of_t][:, None] + cb0[gidx_of_t, :] + toff)  # [NT, NSH]

    # in-degree slot layout for deg computation (original edges only)
    corei = dst0 // NS
    lni = dst0 - corei * NS
    keyd = corei * NS + lni
    cntd = np.bincount(keyd, minlength=NCORES * NS).reshape(NCORES, NS)
    CAPD = int(cntd.max())

    # pooling: first cluster per graph (exact reference semantics)
    ncl = N // 2
    bp = batch[0::2]
    first = np.full(B, np.iinfo(np.int32).max, np.int64)
    np.minimum.at(first, bp, np.arange(ncl, dtype=np.int64))
    cl = np.clip(first, 0, ncl - 1)
    row_even = 2 * cl
    owner = (row_even // NS).astype(np.int64)
    loc_even = row_even - owner * NS

    iota_np = np.broadcast_to(
        np.arange(P, dtype=np.float32), (P, P)).copy()
    b1r = np.broadcast_to(b1.astype(np.float32), (P, D)).copy()
    b2r = np.broadcast_to(b2.astype(np.float32), (P, D)).copy()

    in_maps = []
    for c in range(NCORES):
        m = core == c
        tt = t[m]
        ss = s[m]
        cell = tt * NSH + ss
        order = np.argsort(cell, kind="stable")
        cell_s = cell[order]
        cnt_c = np.bincount(cell, minlength=NT * NSH)
        starts = np.concatenate([[0], np.cumsum(cnt_c)])[:-1]
        rank = np.arange(cell_s.size) - starts[cell_s]
        slot = cellbase.reshape(-1)[cell_s] + rank

        dstl_arr = np.full((P, TC), -1.0, np.float32)
        w_arr = np.zeros((P, TC), np.float32)
        lane = slot % P
        cpos = slot // P
        dstl_arr[lane, cpos] = dl[m][order]
        w_arr[lane, cpos] = w[m][order]

        gidx_arr = np.zeros((P, ICOLS), np.int16)
        inst = gidx_of_t[tt[order]] * NSH + ss[order]
        instr_chunk_base = (gbase[gidx_of_t] [:, None] + cb0[gidx_of_t, :])  # [NT, NSH] -> per (g,s) const
        j = slot - P * instr_chunk_base.reshape(NT, NSH)[tt[order], ss[order]]
        colb = icb.reshape(-1)[inst]
        col = colb + j // 16
        row16 = (j % 16).astype(np.int64)
        liv = li[m][order]
        for rep in range(8):
            gidx_arr[16 * rep + row16, col] = liv

        # degree slots
        md = corei == c
        lnc = lni[md]
        od = np.argsort(lnc, kind="stable")
        lns = lnc[od]
        startsd = np.concatenate([[0], np.cumsum(cntd[c])])[:-1]
        rankd = np.arange(lns.size) - startsd[lns]
        degw_arr = np.zeros((P, NT, CAPD), np.float32)
        degw_arr[lns % P, lns // P, rankd] = edge_weight[md][od]

        # pooling gather indices (512: evens then odds), 0 for non-owned
        pe = np.where(owner == c, loc_even, 0).astype(np.int64)
        po = np.where(owner == c, loc_even + 1, 0).astype(np.int64)
        pidx_flat = np.concatenate([pe, po]).astype(np.int16)
        pidx_arr = np.zeros((P, 32), np.int16)
        jj = np.arange(512)
        for rep in range(8):
            pidx_arr[16 * rep + jj % 16, jj // 16] = pidx_flat

        xT = np.zeros((P, NSP), np.float32)
        xT[:, :NS] = x[c * NS:(c + 1) * NS].T

        in_maps.append({
            "xT": xT,
            "degw": degw_arr.reshape(P, NT * CAPD),
            "dstl": dstl_arr,
            "wst": w_arr,
            "gidx": gidx_arr,
            "pidx": pidx_arr,
            "W1": W1,
            "W2": W2,
            "b1r": b1r,
            "b2r": b2r,
            "iota": iota_np,
        })

    tables = dict(K=K, NIgs=NIgs, cb0=cb0, toff=toff, gbase=gbase, Cg=Cg,
                  icb=icb, TC=TC, ICOLS=ICOLS, CAPD=CAPD)
    return in_maps, tables, owner


def _build(tables):
    import concourse.bass as bass
    import concourse.tile as tile
    from concourse import mybir, bacc, library_config

    K = tables["K"]
    NIgs = tables["NIgs"]
    cb0 = tables["cb0"]
    toff = tables["toff"]
    gbase = tables["gbase"]
    Cg = tables["Cg"]
    icb = tables["icb"]
    TC = tables["TC"]
    ICOLS = tables["ICOLS"]
    CAPD = tables["CAPD"]

    f32 = mybir.dt.float32
    i16 = mybir.dt.int16
    AOP = mybir.AluOpType

    nc = bacc.Bacc("TRN2", target_bir_lowering=False, debug=False,
                   num_devices=NCORES, dynamic_dma_scratch_size=32768)

    xT = nc.declare_dram_parameter("xT", [P, NSP], f32, isOutput=False)
    degw = nc.declare_dram_parameter("degw", [P, NT * CAPD], f32, isOutput=False)
    dstl = nc.declare_dram_parameter("dstl", [P, TC], f32, isOutput=False)
    wst = nc.declare_dram_parameter("wst", [P, TC], f32, isOutput=False)
    gidx = nc.declare_dram_parameter("gidx", [P, ICOLS], i16, isOutput=False)
    pidx = nc.declare_dram_parameter("pidx", [P, 32], i16, isOutput=False)
    W1 = nc.declare_dram_parameter("W1", [IN_DIM, D], f32, isOutput=False)
    W2 = nc.declare_dram_parameter("W2", [D, D], f32, isOutput=False)
    b1r = nc.declare_dram_parameter("b1r", [P, D], f32, isOutput=False)
    b2r = nc.declare_dram_parameter("b2r", [P, D], f32, isOutput=False)
    iota = nc.declare_dram_parameter("iota", [P, P], f32, isOutput=False)
    pool_out = nc.declare_dram_parameter("pool_out", [P, 2, D], f32, isOutput=True)

    tab1_mine = nc.dram_tensor("tab1_mine", [NSP, D], f32)
    tab2_mine = nc.dram_tensor("tab2_mine", [NSP, D], f32)
    tab1_full = nc.dram_tensor("tab1_full", [TOTR, D], f32, addr_space="Shared")
    tab2_full = nc.dram_tensor("tab2_full", [TOTR, D], f32, addr_space="Shared")
    h2_local = nc.dram_tensor("h2_local", [NSP, D], f32)

    groups = [list(range(NCORES))]

    # per-tile group-local chunk positions
    tile_chunks = []
    for t in range(NT):
        g = t // GT
        lst = []
        for s in range(NSH):
            base = cb0[g, s] + toff[t, s]
            for k in range(int(K[t, s])):
                lst.append(int(base + k))
        tile_chunks.append(lst)

    from contextlib import ExitStack
    with ExitStack() as top:
        tc = top.enter_context(tile.TileContext(nc))
        nc.gpsimd.load_library(library_config.mlp)
        const = top.enter_context(tc.tile_pool(name="const", bufs=1))
        iota_t = const.tile([P, P], f32)
        nc.sync.dma_start(out=iota_t[:], in_=iota[:])
        W1_t = const.tile([IN_DIM, D], f32)
        nc.sync.dma_start(out=W1_t[:], in_=W1[:])
        W2_t = const.tile([D, D], f32)
        nc.sync.dma_start(out=W2_t[:], in_=W2[:])
        b1r_t = const.tile([P, D], f32)
        nc.sync.dma_start(out=b1r_t[:], in_=b1r[:])
        b2r_t = const.tile([P, D], f32)
        nc.sync.dma_start(out=b2r_t[:], in_=b2r[:])
        from concourse.masks import make_identity
        ident = const.tile([P, P], f32)
        make_identity(nc, ident[:])
        dinv = const.tile([P, NT], f32)

        # ---- deg / dinv ----
        with tc.tile_pool(name="degp", bufs=1) as degp:
            degw_t = degp.tile([P, NT, CAPD], f32)
            nc.sync.dma_start(out=degw_t[:],
                              in_=degw[:].rearrange_free_dims([NT, CAPD]) if hasattr(degw[:], "rearrange_free_dims") else degw[:])
            deg = degp.tile([P, NT], f32)
            nc.vector.tensor_reduce(out=deg[:], in_=degw_t[:],
                                    axis=mybir.AxisListType.X, op=AOP.add)
            degq = degp.tile([P, NT], f32)
            nc.vector.tensor_scalar_add(out=degq[:], in0=deg[:], scalar1=1.0)
            dsq = degp.tile([P, NT], f32)
            nc.scalar.sqrt(out=dsq[:], in_=degq[:])
            nc.vector.reciprocal(out=dinv[:], in_=dsq[:])

        # ---- layer 1 xw: tab1 = dinv * (x @ W1) ----
        with tc.tile_pool(name="xwp", bufs=3) as xwp, \
             tc.tile_pool(name="xwps", bufs=2, space="PSUM") as xwps:
            for t in range(NT):
                xT_t = xwp.tile([P, P], f32, tag="xTt")
                nc.sync.dma_start(out=xT_t[:], in_=xT[:, t * P:(t + 1) * P])
                psx = xwps.tile([P, D], f32, tag="xw1")
                nc.tensor.matmul(out=psx[:], lhsT=xT_t[:], rhs=W1_t[:],
                                 start=True, stop=True)
                tabt = xwp.tile([P, D], f32, tag="tabt")
                nc.vector.tensor_scalar_mul(out=tabt[:], in0=psx[:],
                                            scalar1=dinv[:, t:t + 1])
                nc.sync.dma_start(out=tab1_mine[t * P:(t + 1) * P, :], in_=tabt[:])

        nc.gpsimd.collective_compute(
            "AllGather", AOP.bypass, replica_groups=groups,
            ins=[tab1_mine[:]], outs=[tab1_full[:]])

        # ---- edge passes ----
        def edge_pass(layer, tabsrc):
            with tc.tile_pool(name=f"ep{layer}", bufs=2) as ep, \
                 tc.tile_pool(name=f"sel{layer}", bufs=4) as selp, \
                 tc.tile_pool(name=f"fin{layer}", bufs=3) as finp, \
                 tc.tile_pool(name=f"eps{layer}", bufs=3, space="PSUM") as epsp, \
                 tc.tile_pool(name=f"fps{layer}", bufs=2, space="PSUM") as fpsp:
                for g in range(NG):
                    cg = int(Cg[g])
                    gb = int(gbase[g])
                    dstl_t = ep.tile([P, cg], f32, tag="dstl")
                    nc.scalar.dma_start(out=dstl_t[:], in_=dstl[:, gb:gb + cg])
                    w_t = ep.tile([P, cg], f32, tag="wt")
                    nc.scalar.dma_start(out=w_t[:], in_=wst[:, gb:gb + cg])
                    ic0 = int(icb[g, 0])
                    icn = int(NIgs[g].sum() // 16)
                    idx_t = ep.tile([P, icn], i16, tag="idx")
                    nc.scalar.dma_start(out=idx_t[:], in_=gidx[:, ic0:ic0 + icn])
                    gbuf = ep.tile([P, cg, D], f32, tag="gbuf")
                    for s in range(NSH):
                        ni = int(NIgs[g, s])
                        if ni == 0:
                            continue
                        c0 = int(cb0[g, s])
                        nchunk = ni // P
                        il0 = int(icb[g, s]) - ic0
                        nc.gpsimd.dma_gather(
                            gbuf[:, c0:c0 + nchunk, :],
                            tabsrc[s * SHR:(s + 1) * SHR, :],
                            idx_t[:, il0:il0 + ni // 16],
                            ni, ni, D, single_packet=False)
                    for tloc in range(GT):
                        t = g * GT + tloc
                        chunks = tile_chunks[t]
                        ps_t = epsp.tile([P, D], f32, tag="eps")
                        for j, cp in enumerate(chunks):
                            sel = selp.tile([P, P], f32, tag="sel")
                            nc.vector.tensor_scalar(
                                out=sel[:], in0=iota_t[:],
                                scalar1=dstl_t[:, cp:cp + 1],
                                scalar2=w_t[:, cp:cp + 1],
                                op0=AOP.is_equal, op1=AOP.mult)
                            nc.tensor.matmul(out=ps_t[:], lhsT=sel[:],
                                             rhs=gbuf[:, cp, :],
                                             start=(j == 0),
                                             stop=(j == len(chunks) - 1))
                        tmp = finp.tile([P, D], f32, tag="tmp")
                        nc.vector.tensor_scalar_mul(out=tmp[:], in0=ps_t[:],
                                                    scalar1=dinv[:, t:t + 1])
                        if layer == 1:
                            h1 = finp.tile([P, D], f32, tag="h1")
                            nc.vector.tensor_tensor(out=h1[:], in0=tmp[:],
                                                    in1=b1r_t[:], op=AOP.add)
                            psT = fpsp.tile([D, P], f32, tag="tps")
                            nc.tensor.transpose(out=psT[:], in_=h1[:],
                                                identity=ident[:])
                            h1T = finp.tile([D, P], f32, tag="h1T")
                            nc.vector.tensor_copy(out=h1T[:], in_=psT[:])
                            psx2 = fpsp.tile([P, D], f32, tag="xw2")
                            nc.tensor.matmul(out=psx2[:], lhsT=h1T[:],
                                             rhs=W2_t[:], start=True, stop=True)
                            tab2t = finp.tile([P, D], f32, tag="tab2t")
                            nc.vector.tensor_scalar_mul(out=tab2t[:], in0=psx2[:],
                                                        scalar1=dinv[:, t:t + 1])
                            nc.sync.dma_start(out=tab2_mine[t * P:(t + 1) * P, :],
                                              in_=tab2t[:])
                        else:
                            h2 = finp.tile([P, D], f32, tag="h2")
                            nc.vector.tensor_tensor(out=h2[:], in0=tmp[:],
                                                    in1=b2r_t[:], op=AOP.add)
                            nc.sync.dma_start(out=h2_local[t * P:(t + 1) * P, :],
                                              in_=h2[:])

        edge_pass(1, tab1_full)
        nc.gpsimd.collective_compute(
            "AllGather", AOP.bypass, replica_groups=groups,
            ins=[tab2_mine[:]], outs=[tab2_full[:]])
        edge_pass(2, tab2_full)

        # ---- pooling head ----
        with tc.tile_pool(name="poolp", bufs=1) as pp:
            pidx_t = pp.tile([P, 32], i16)
            nc.sync.dma_start(out=pidx_t[:], in_=pidx[:])
            pbuf = pp.tile([P, 4, D], f32)
            nc.gpsimd.dma_gather(pbuf[:], h2_local[:], pidx_t[:], 512, 512, D,
                                 single_packet=False)
            pm = pp.tile([P, 2, D], f32)
            nc.vector.tensor_tensor(out=pm[:], in0=pbuf[:, 0:2, :],
                                    in1=pbuf[:, 2:4, :], op=AOP.max)
            nc.sync.dma_start(out=pool_out[:], in_=pm[:])

    nc.compile()
    return nc


LAST_RESULTS = None


def kernel(**inputs):
    global LAST_RESULTS
    from concourse.bass_utils import run_bass_kernel_spmd

    in_maps, tables, owner = _prepare(inputs)
    nc = _build(tables)
    res = run_bass_kernel_spmd(nc, in_maps, list(range(NCORES)))
    LAST_RESULTS = res
    out = np.zeros((B, D), np.float32)
    bb = np.arange(B)
    for c in range(NCORES):
        m = owner == c
        if m.any():
            po = res.results[c]["pool_out"]
            out[bb[m]] = po[bb[m] % P, bb[m] // P, :]
    return out



# revision 8
# speedup vs baseline: 1.0036x; 1.0036x over previous
"""GCN (2x GCNConv + graclus-style max-pool head) on 8 Trainium2 NeuronCores.

Strategy (graph partitioning per the sharding hint):
  - Nodes are sharded contiguously across 8 cores (12500 each, padded to
    12544 = 98 tiles of 128).  Edges are partitioned by destination node.
  - deg / dinv = 1/sqrt(deg) are computed fully locally (all edges with a
    given dst live on its owner core).
  - Per layer: each core computes dinv * (x_shard @ W) locally, then an
    8-rank AllGather replicates the full [100352, 64] feature table.
  - Edge pass: per 128-edge chunk, dma_gather pulls table[src] rows into
    SBUF, a selection matrix sel[e, n] = (iota[n] == dst_local[e]) * w[e]
    is built with one fused tensor_scalar op, and the TensorEngine
    accumulates sel.T @ gathered into the per-tile PSUM (segment-sum).
    Self loops are included as ordinary edges with w = 1.
  - Pooling head: out[b] = max(h2[2c], h2[2c+1]) for the first cluster c of
    graph b; those 512 rows are fetched with one small dma_gather and
    reduced with a single elementwise max.
"""

import os
import sys

sys.path.insert(0, "/opt/trn_rl_repo")

import numpy as np

N = 100000
E = 1600000
B = 256
IN_DIM = 128
OUT_DIM = 64
NCORES = 8
NS = N // NCORES          # 12500 real nodes per core
NT = (NS + 127) // 128    # 98 tiles per core
NSP = NT * 128            # 12544 padded nodes per core
GT = 7                    # tiles per gather group
NG = NT // GT             # 14 groups
NSH = 4                   # src table shards (int16 gather index limit)
SHR = 2 * NSP             # 25088 rows per shard
TOTR = NCORES * NSP       # 100352 table rows
P = 128
D = OUT_DIM


def _prepare(inputs):
    x = np.asarray(inputs["x"], dtype=np.float32)
    edge_index = np.asarray(inputs["edge_index"], dtype=np.int64)
    edge_weight = np.asarray(inputs["edge_weight"], dtype=np.float32)
    batch = np.asarray(inputs["batch"], dtype=np.int64)
    W1 = np.asarray(inputs["W1"], dtype=np.float32)
    b1 = np.asarray(inputs["b1"], dtype=np.float32)
    W2 = np.asarray(inputs["W2"], dtype=np.float32)
    b2 = np.asarray(inputs["b2"], dtype=np.float32)

    src0 = edge_index[0]
    dst0 = edge_index[1]
    loop = np.arange(N, dtype=np.int64)
    src = np.concatenate([src0, loop])
    dst = np.concatenate([dst0, loop])
    w = np.concatenate([edge_weight, np.ones(N, np.float32)])

    core = dst // NS
    lt = dst - core * NS            # local node id 0..12499
    t = lt // P                     # tile 0..97
    dl = (lt - t * P).astype(np.float32)
    r = (src // NS) * NSP + (src % NS)   # padded table row
    s = r // SHR                    # src shard 0..3
    li = (r - s * SHR).astype(np.int16)

    # per (core, tile, shard) counts -> global chunk capacities K[t, s]
    key = ((core * NT + t) * NSH + s).astype(np.int64)
    cnt = np.bincount(key, minlength=NCORES * NT * NSH).reshape(NCORES, NT, NSH)
    K = ((cnt.max(axis=0) + P - 1) // P).astype(np.int64)   # [NT, NSH]

    # group-local layout: for group g: for s: for t in g: K[t,s] chunks
    cb0 = np.zeros((NG, NSH), np.int64)       # group-local chunk base of (g, s)
    toff = np.zeros((NT, NSH), np.int64)      # chunk offset of tile within (g, s)
    NIgs = np.zeros((NG, NSH), np.int64)      # idxs per gather instruction
    Cg = np.zeros(NG, np.int64)               # chunks per group
    for g in range(NG):
        tl = range(g * GT, (g + 1) * GT)
        off = 0
        for sh in range(NSH):
            cb0[g, sh] = off
            o2 = 0
            for tt in tl:
                toff[tt, sh] = o2
                o2 += K[tt, sh]
            NIgs[g, sh] = P * o2
            off += o2
        Cg[g] = off
    gbase = np.concatenate([[0], np.cumsum(Cg)])   # group chunk base, [NG+1]
    TC = int(gbase[-1])                            # total chunks per layer
    icb = np.zeros((NG, NSH), np.int64)            # idx col base per instruction
    run = 0
    for g in range(NG):
        for sh in range(NSH):
            icb[g, sh] = run
            run += NIgs[g, sh] // 16
    ICOLS = int(run)

    # global slot base of each (t, s) cell
    gidx_of_t = np.arange(NT) // GT
    cellbase = P * (gbase[gidx_of_t][:, None] + cb0[gidx_of_t, :] + toff)  # [NT, NSH]

    # in-degree slot layout for deg computation (original edges only)
    corei = dst0 // NS
    lni = dst0 - corei * NS
    keyd = corei * NS + lni
    cntd = np.bincount(keyd, minlength=NCORES * NS).reshape(NCORES, NS)
    CAPD = int(cntd.max())

    # pooling: first cluster per graph (exact reference semantics)
    ncl = N // 2
    bp = batch[0::2]
    first = np.full(B, np.iinfo(np.int32).max, np.int64)
    np.minimum.at(first, bp, np.arange(ncl, dtype=np.int64))
    cl = np.clip(first, 0, ncl - 1)
    row_even = 2 * cl
    owner = (row_even // NS).astype(np.int64)
    loc_even = row_even - owner * NS

    iota_np = np.broadcast_to(
        np.arange(P, dtype=np.float32), (P, P)).copy()
    b1r = np.broadcast_to(b1.astype(np.float32), (P, D)).copy()
    b2r = np.broadcast_to(b2.astype(np.float32), (P, D)).copy()

    in_maps = []
    for c in range(NCORES):
        m = core == c
        tt = t[m]
        ss = s[m]
        cell = tt * NSH + ss
        order = np.argsort(cell, kind="stable")
        cell_s = cell[order]
        cnt_c = np.bincount(cell, minlength=NT * NSH)
        starts = np.concatenate([[0], np.cumsum(cnt_c)])[:-1]
        rank = np.arange(cell_s.size) - starts[cell_s]
        slot = cellbase.reshape(-1)[cell_s] + rank

        dstl_arr = np.full((P, TC), -1.0, np.float32)
        w_arr = np.zeros((P, TC), np.float32)
        lane = slot % P
        cpos = slot // P
        dstl_arr[lane, cpos] = dl[m][order]
        w_arr[lane, cpos] = w[m][order]

        gidx_arr = np.zeros((P, ICOLS), np.int16)
        inst = gidx_of_t[tt[order]] * NSH + ss[order]
        instr_chunk_base = (gbase[gidx_of_t] [:, None] + cb0[gidx_of_t, :])  # [NT, NSH] -> per (g,s) const
        j = slot - P * instr_chunk_base.reshape(NT, NSH)[tt[order], ss[order]]
        colb = icb.reshape(-1)[inst]
        col = colb + j // 16
        row16 = (j % 16).astype(np.int64)
        liv = li[m][order]
        for rep in range(8):
            gidx_arr[16 * rep + row16, col] = liv

        # degree slots
        md = corei == c
        lnc = lni[md]
        od = np.argsort(lnc, kind="stable")
        lns = lnc[od]
        startsd = np.concatenate([[0], np.cumsum(cntd[c])])[:-1]
        rankd = np.arange(lns.size) - startsd[lns]
        degw_arr = np.zeros((P, NT, CAPD), np.float32)
        degw_arr[lns % P, lns // P, rankd] = edge_weight[md][od]

        # pooling gather indices (512: evens then odds), 0 for non-owned
        pe = np.where(owner == c, loc_even, 0).astype(np.int64)
        po = np.where(owner == c, loc_even + 1, 0).astype(np.int64)
        pidx_flat = np.concatenate([pe, po]).astype(np.int16)
        pidx_arr = np.zeros((P, 32), np.int16)
        jj = np.arange(512)
        for rep in range(8):
            pidx_arr[16 * rep + jj % 16, jj // 16] = pidx_flat

        xT = np.zeros((P, NSP), np.float32)
        xT[:, :NS] = x[c * NS:(c + 1) * NS].T

        in_maps.append({
            "xT": xT,
            "degw": degw_arr.reshape(P, NT * CAPD),
            "dstl": dstl_arr,
            "wst": w_arr,
            "gidx": gidx_arr,
            "pidx": pidx_arr,
            "W1": W1,
            "W2": W2,
            "b1r": b1r,
            "b2r": b2r,
            "iota": iota_np,
        })

    tables = dict(K=K, NIgs=NIgs, cb0=cb0, toff=toff, gbase=gbase, Cg=Cg,
                  icb=icb, TC=TC, ICOLS=ICOLS, CAPD=CAPD)
    return in_maps, tables, owner


def _build(tables):
    import concourse.bass as bass
    import concourse.tile as tile
    from concourse import mybir, bacc, library_config

    K = tables["K"]
    NIgs = tables["NIgs"]
    cb0 = tables["cb0"]
    toff = tables["toff"]
    gbase = tables["gbase"]
    Cg = tables["Cg"]
    icb = tables["icb"]
    TC = tables["TC"]
    ICOLS = tables["ICOLS"]
    CAPD = tables["CAPD"]

    f32 = mybir.dt.float32
    i16 = mybir.dt.int16
    AOP = mybir.AluOpType

    nc = bacc.Bacc("TRN2", target_bir_lowering=False, debug=False,
                   num_devices=NCORES, dynamic_dma_scratch_size=32768)

    xT = nc.declare_dram_parameter("xT", [P, NSP], f32, isOutput=False)
    degw = nc.declare_dram_parameter("degw", [P, NT * CAPD], f32, isOutput=False)
    dstl = nc.declare_dram_parameter("dstl", [P, TC], f32, isOutput=False)
    wst = nc.declare_dram_parameter("wst", [P, TC], f32, isOutput=False)
    gidx = nc.declare_dram_parameter("gidx", [P, ICOLS], i16, isOutput=False)
    pidx = nc.declare_dram_parameter("pidx", [P, 32], i16, isOutput=False)
    W1 = nc.declare_dram_parameter("W1", [IN_DIM, D], f32, isOutput=False)
    W2 = nc.declare_dram_parameter("W2", [D, D], f32, isOutput=False)
    b1r = nc.declare_dram_parameter("b1r", [P, D], f32, isOutput=False)
    b2r = nc.declare_dram_parameter("b2r", [P, D], f32, isOutput=False)
    iota = nc.declare_dram_parameter("iota", [P, P], f32, isOutput=False)
    pool_out = nc.declare_dram_parameter("pool_out", [P, 2, D], f32, isOutput=True)

    tab1_mine = nc.dram_tensor("tab1_mine", [NSP, D], f32)
    tab2_mine = nc.dram_tensor("tab2_mine", [NSP, D], f32)
    tab1_full = nc.dram_tensor("tab1_full", [TOTR, D], f32, addr_space="Shared")
    tab2_full = nc.dram_tensor("tab2_full", [TOTR, D], f32, addr_space="Shared")
    h2_local = nc.dram_tensor("h2_local", [NSP, D], f32)

    groups = [list(range(NCORES))]

    # per-tile group-local chunk positions
    tile_chunks = []
    for t in range(NT):
        g = t // GT
        lst = []
        for s in range(NSH):
            base = cb0[g, s] + toff[t, s]
            for k in range(int(K[t, s])):
                lst.append(int(base + k))
        tile_chunks.append(lst)

    from contextlib import ExitStack
    with ExitStack() as top:
        tc = top.enter_context(tile.TileContext(nc))
        nc.gpsimd.load_library(library_config.mlp)
        const = top.enter_context(tc.tile_pool(name="const", bufs=1))
        iota_t = const.tile([P, P], f32)
        nc.sync.dma_start(out=iota_t[:], in_=iota[:])
        W1_t = const.tile([IN_DIM, D], f32)
        nc.sync.dma_start(out=W1_t[:], in_=W1[:])
        W2_t = const.tile([D, D], f32)
        nc.sync.dma_start(out=W2_t[:], in_=W2[:])
        b1r_t = const.tile([P, D], f32)
        nc.sync.dma_start(out=b1r_t[:], in_=b1r[:])
        b2r_t = const.tile([P, D], f32)
        nc.sync.dma_start(out=b2r_t[:], in_=b2r[:])
        from concourse.masks import make_identity
        ident = const.tile([P, P], f32)
        make_identity(nc, ident[:])
        dinv = const.tile([P, NT], f32)

        # ---- deg / dinv ----
        with tc.tile_pool(name="degp", bufs=1) as degp:
            degw_t = degp.tile([P, NT, CAPD], f32)
            nc.sync.dma_start(out=degw_t[:],
                              in_=degw[:].rearrange_free_dims([NT, CAPD]) if hasattr(degw[:], "rearrange_free_dims") else degw[:])
            deg = degp.tile([P, NT], f32)
            nc.vector.tensor_reduce(out=deg[:], in_=degw_t[:],
                                    axis=mybir.AxisListType.X, op=AOP.add)
            degq = degp.tile([P, NT], f32)
            nc.vector.tensor_scalar_add(out=degq[:], in0=deg[:], scalar1=1.0)
            dsq = degp.tile([P, NT], f32)
            nc.scalar.sqrt(out=dsq[:], in_=degq[:])
            nc.vector.reciprocal(out=dinv[:], in_=dsq[:])

        # ---- layer 1 xw: tab1 = dinv * (x @ W1) ----
        with tc.tile_pool(name="xwp", bufs=3) as xwp, \
             tc.tile_pool(name="xwps", bufs=2, space="PSUM") as xwps:
            for t in range(NT):
                xT_t = xwp.tile([P, P], f32, tag="xTt")
                nc.sync.dma_start(out=xT_t[:], in_=xT[:, t * P:(t + 1) * P])
                psx = xwps.tile([P, D], f32, tag="xw1")
                nc.tensor.matmul(out=psx[:], lhsT=xT_t[:], rhs=W1_t[:],
                                 start=True, stop=True)
                tabt = xwp.tile([P, D], f32, tag="tabt")
                nc.vector.tensor_scalar_mul(out=tabt[:], in0=psx[:],
                                            scalar1=dinv[:, t:t + 1])
                nc.sync.dma_start(out=tab1_mine[t * P:(t + 1) * P, :], in_=tabt[:])

        nc.gpsimd.collective_compute(
            "AllGather", AOP.bypass, replica_groups=groups,
            ins=[tab1_mine[:]], outs=[tab1_full[:]])

        # ---- edge passes ----
        def edge_pass(layer, tabsrc):
            with tc.tile_pool(name=f"ep{layer}", bufs=2) as ep, \
                 tc.tile_pool(name=f"sel{layer}", bufs=4) as selp, \
                 tc.tile_pool(name=f"fin{layer}", bufs=3) as finp, \
                 tc.tile_pool(name=f"eps{layer}", bufs=3, space="PSUM") as epsp, \
                 tc.tile_pool(name=f"fps{layer}", bufs=2, space="PSUM") as fpsp:
                for g in range(NG):
                    cg = int(Cg[g])
                    gb = int(gbase[g])
                    dstl_t = ep.tile([P, cg], f32, tag="dstl")
                    nc.scalar.dma_start(out=dstl_t[:], in_=dstl[:, gb:gb + cg])
                    w_t = ep.tile([P, cg], f32, tag="wt")
                    nc.scalar.dma_start(out=w_t[:], in_=wst[:, gb:gb + cg])
                    ic0 = int(icb[g, 0])
                    icn = int(NIgs[g].sum() // 16)
                    idx_t = ep.tile([P, icn], i16, tag="idx")
                    nc.scalar.dma_start(out=idx_t[:], in_=gidx[:, ic0:ic0 + icn])
                    gbuf = ep.tile([P, cg, D], f32, tag="gbuf")
                    for s in range(NSH):
                        ni = int(NIgs[g, s])
                        if ni == 0:
                            continue
                        c0 = int(cb0[g, s])
                        nchunk = ni // P
                        il0 = int(icb[g, s]) - ic0
                        nc.gpsimd.dma_gather(
                            gbuf[:, c0:c0 + nchunk, :],
                            tabsrc[s * SHR:(s + 1) * SHR, :],
                            idx_t[:, il0:il0 + ni // 16],
                            ni, ni, D, single_packet=False)
                    for tloc in range(GT):
                        t = g * GT + tloc
                        chunks = tile_chunks[t]
                        ps_t = epsp.tile([P, D], f32, tag="eps")
                        for j, cp in enumerate(chunks):
                            sel = selp.tile([P, P], f32, tag="sel")
                            nc.vector.tensor_scalar(
                                out=sel[:], in0=iota_t[:],
                                scalar1=dstl_t[:, cp:cp + 1],
                                scalar2=w_t[:, cp:cp + 1],
                                op0=AOP.is_equal, op1=AOP.mult)
                            nc.tensor.matmul(out=ps_t[:], lhsT=sel[:],
                                             rhs=gbuf[:, cp, :],
                                             start=(j == 0),
                                             stop=(j == len(chunks) - 1))
                        tmp = finp.tile([P, D], f32, tag="tmp")
                        nc.vector.tensor_scalar_mul(out=tmp[:], in0=ps_t[:],
                                                    scalar1=dinv[:, t:t + 1])
                        if layer == 1:
                            h1 = finp.tile([P, D], f32, tag="h1")
                            nc.vector.tensor_tensor(out=h1[:], in0=tmp[:],
                                                    in1=b1r_t[:], op=AOP.add)
                            psT = fpsp.tile([D, P], f32, tag="tps")
                            nc.tensor.transpose(out=psT[:], in_=h1[:],
                                                identity=ident[:])
                            h1T = finp.tile([D, P], f32, tag="h1T")
                            nc.vector.tensor_copy(out=h1T[:], in_=psT[:])
                            psx2 = fpsp.tile([P, D], f32, tag="xw2")
                            nc.tensor.matmul(out=psx2[:], lhsT=h1T[:],
                                             rhs=W2_t[:], start=True, stop=True)
                            tab2t = finp.tile([P, D], f32, tag="tab2t")
                            nc.vector.tensor_scalar_mul(out=tab2t[:], in0=psx2[:],
                                                        scalar1=dinv[:, t:t + 1])
                            nc.sync.dma_start(out=tab2_mine[t * P:(t + 1) * P, :],
                                              in_=tab2t[:])
                        else:
                            h2 = finp.tile([P, D], f32, tag="h2")
                            nc.vector.tensor_tensor(out=h2[:], in0=tmp[:],
                                                    in1=b2r_t[:], op=AOP.add)
                            nc.sync.dma_start(out=h2_local[t * P:(t + 1) * P, :],
                                              in_=h2[:])

        edge_pass(1, tab1_full)
        nc.gpsimd.collective_compute(
            "AllGather", AOP.bypass, replica_groups=groups,
            ins=[tab2_mine[:]], outs=[tab2_full[:]])
        edge_pass(2, tab2_full)

        # ---- pooling head ----
        with tc.tile_pool(name="poolp", bufs=1) as pp:
            pidx_t = pp.tile([P, 32], i16)
            nc.sync.dma_start(out=pidx_t[:], in_=pidx[:])
            pbuf = pp.tile([P, 4, D], f32)
            nc.gpsimd.dma_gather(pbuf[:], h2_local[:], pidx_t[:], 512, 512, D,
                                 single_packet=False)
            pm = pp.tile([P, 2, D], f32)
            nc.vector.tensor_tensor(out=pm[:], in0=pbuf[:, 0:2, :],
                                    in1=pbuf[:, 2:4, :], op=AOP.max)
            nc.sync.dma_start(out=pool_out[:], in_=pm[:])

    nc.compile()
    return nc


LAST_RESULTS = None


def kernel(**inputs):
    global LAST_RESULTS
    from concourse.bass_utils import run_bass_kernel_spmd

    in_maps, tables, owner = _prepare(inputs)
    nc = _build(tables)
    res = run_bass_kernel_spmd(nc, in_maps, list(range(NCORES)))
    LAST_RESULTS = res
    out = np.zeros((B, D), np.float32)
    bb = np.arange(B)
    for c in range(NCORES):
        m = owner == c
        if m.any():
            po = res.results[c]["pool_out"]
            out[bb[m]] = po[bb[m] % P, bb[m] // P, :]
    return out



# revision 10
# speedup vs baseline: 24.9512x; 24.8607x over previous
"""GCN (2x GCNConv + graclus-style max-pool head) on 8 Trainium2 NeuronCores.

v2 strategy — prune + linearity + host-built routing:
  - The output reads h2 at only 512 nodes (first cluster pair per graph).
    Layer-2 aggregation is computed only for those dst nodes (~8.7K edges),
    and layer-1 aggregation only for the ~8.4K src nodes feeding them
    (~150K edges) — a ~20x cut vs the full 2x1.6M edge passes.
  - Linearity: sum_e norm_e * (x@W1)[src] == (sum_e norm_e * x[src]) @ W1.
    Each core gathers raw x rows (shipped compact + bf16 per core), so the
    dense x@W1 phase and the 25.6MB AllGather disappear entirely.
  - The GCN normalization dinv[src]*w*dinv[dst] is folded into host-built
    per-chunk selection matrices sel[e, lane] (one TensorE matmul per
    128-edge chunk accumulates the segment-sum in PSUM).
  - Edges partitioned by dst owner core; per dst-tile chunk capacities are
    maxed across cores so all 8 cores run one SPMD program.
  - Layer 1 per tile:  xaggT = sum_k gbx_k^T @ sel_k   [128 xdim, 128 nodes]
                       h1T   = W1^T @ xaggT + b1       [64, 128]
                       tab2  = h1T^T @ W2              [128, 64] -> AllGather
  - Layer 2 per pool tile (64 graphs: lanes 0-63 = even pair member,
    64-127 = odd):     h2T = sum_k gb2_k^T @ sel2_k    [64, 128]
                       out = max(h2T[:, :64], h2T[:, 64:]) + b2
"""

import sys

sys.path.insert(0, "/opt/trn_rl_repo")

import numpy as np
import ml_dtypes

N = 100000
E = 1600000
B = 256
IN_DIM = 128
OUT_DIM = 64
NCORES = 8
NS = N // NCORES
P = 128
D = OUT_DIM


def _prepare(inputs):
    x = np.asarray(inputs["x"], dtype=np.float32)
    edge_index = np.asarray(inputs["edge_index"]).astype(np.int64)
    ew = np.asarray(inputs["edge_weight"], dtype=np.float32)
    batch = np.asarray(inputs["batch"]).astype(np.int64)
    W1 = np.asarray(inputs["W1"], dtype=np.float32)
    b1 = np.asarray(inputs["b1"], dtype=np.float32)
    W2 = np.asarray(inputs["W2"], dtype=np.float32)
    b2 = np.asarray(inputs["b2"], dtype=np.float32)

    src0, dst0 = edge_index[0], edge_index[1]
    deg = np.zeros(N, np.float64)
    np.add.at(deg, dst0, ew.astype(np.float64))
    deg += 1.0
    dinv = (1.0 / np.sqrt(deg)).astype(np.float32)

    srcA = np.concatenate([src0, np.arange(N, dtype=np.int64)])
    dstA = np.concatenate([dst0, np.arange(N, dtype=np.int64)])
    wA = np.concatenate([ew, np.ones(N, np.float32)])
    norm = dinv[srcA] * wA * dinv[dstA]

    # CSR by dst
    order = np.argsort(dstA, kind="stable")
    srcS = srcA[order]
    normS = norm[order]
    dcnt = np.bincount(dstA, minlength=N)
    dstart = np.zeros(N + 1, np.int64)
    np.cumsum(dcnt, out=dstart[1:])

    # pooling head: first cluster of each graph
    ncl = N // 2
    bp = batch[0::2]
    first = np.full(B, np.iinfo(np.int64).max)
    np.minimum.at(first, bp, np.arange(ncl, dtype=np.int64))
    cl = np.clip(first, 0, ncl - 1)
    evens = 2 * cl
    odds = evens + 1
    gown = evens // NS

    gs = [np.nonzero(gown == c)[0] for c in range(NCORES)]
    T2 = max(1, max((len(g) + 63) // 64 for g in gs))

    # ---- layer-2 edge sets: per (core, pool tile): (src, lane, norm) ----
    e2 = [[None] * T2 for _ in range(NCORES)]
    for c in range(NCORES):
        for j in range(T2):
            gsel = gs[c][j * 64:(j + 1) * 64]
            ss, ll, nn = [], [], []
            for pos, g in enumerate(gsel):
                for parity, v in ((0, evens[g]), (1, odds[g])):
                    s0, s1 = dstart[v], dstart[v + 1]
                    ss.append(srcS[s0:s1])
                    nn.append(normS[s0:s1])
                    ll.append(np.full(s1 - s0, 64 * parity + pos, np.int64))
            if ss:
                e2[c][j] = (np.concatenate(ss), np.concatenate(ll),
                            np.concatenate(nn))
            else:
                e2[c][j] = (np.zeros(0, np.int64), np.zeros(0, np.int64),
                            np.zeros(0, np.float32))

    # ---- S2: distinct srcs over all layer-2 edges; packed by owner ----
    allsrc2 = np.concatenate([e2[c][j][0] for c in range(NCORES)
                              for j in range(T2)])
    S2 = np.unique(allsrc2)
    S2_c = [S2[S2 // NS == c] for c in range(NCORES)]
    T1 = max(1, max((len(s) + P - 1) // P for s in S2_c))
    tab2row = np.full(N, -1, np.int64)
    for c in range(NCORES):
        tab2row[S2_c[c]] = c * T1 * P + np.arange(len(S2_c[c]))
    TAB2R = NCORES * T1 * P
    assert TAB2R <= 32767, TAB2R

    # ---- layer-1 edge sets: per (core, tile): (src, lane, norm) ----
    e1 = [[None] * T1 for _ in range(NCORES)]
    for c in range(NCORES):
        for t in range(T1):
            nodes = S2_c[c][t * P:(t + 1) * P]
            ss, ll, nn = [], [], []
            for lane, v in enumerate(nodes):
                s0, s1 = dstart[v], dstart[v + 1]
                ss.append(srcS[s0:s1])
                nn.append(normS[s0:s1])
                ll.append(np.full(s1 - s0, lane, np.int64))
            if ss:
                e1[c][t] = (np.concatenate(ss), np.concatenate(ll),
                            np.concatenate(nn))
            else:
                e1[c][t] = (np.zeros(0, np.int64), np.zeros(0, np.int64),
                            np.zeros(0, np.float32))

    # chunk capacities (uniform across cores for SPMD)
    K1 = [max(1, max((len(e1[c][t][0]) + P - 1) // P for c in range(NCORES)))
          for t in range(T1)]
    cb1 = np.concatenate([[0], np.cumsum(K1)]).astype(np.int64)
    C1 = int(cb1[-1])
    K2 = [max(1, max((len(e2[c][j][0]) + P - 1) // P for c in range(NCORES)))
          for j in range(T2)]
    cb2 = np.concatenate([[0], np.cumsum(K2)]).astype(np.int64)
    C2 = int(cb2[-1])

    # compact x tables per core
    xsrc = [np.unique(np.concatenate([e1[c][t][0] for t in range(T1)]))
            for c in range(NCORES)]
    XROWS = max(len(s) for s in xsrc)
    assert XROWS <= 32767, XROWS

    in_maps = []
    for c in range(NCORES):
        xg = np.zeros((XROWS, IN_DIM), np.float32)
        xg[:len(xsrc[c])] = x[xsrc[c]]

        sel1 = np.zeros((P, C1, P), np.float32)
        idx1 = np.zeros((P, C1 * 8), np.int16)
        for t in range(T1):
            ss, ll, nn = e1[c][t]
            j = np.arange(len(ss))
            sel1[j % P, cb1[t] + j // P, ll] = nn
            xrow = np.searchsorted(xsrc[c], ss).astype(np.int16)
            col = cb1[t] * 8 + j // 16
            for rep in range(8):
                idx1[16 * rep + j % 16, col] = xrow

        sel2 = np.zeros((P, C2, P), np.float32)
        idx2 = np.zeros((P, C2 * 8), np.int16)
        for t in range(T2):
            ss, ll, nn = e2[c][t]
            j = np.arange(len(ss))
            sel2[j % P, cb2[t] + j // P, ll] = nn
            trow = tab2row[ss]
            assert (trow >= 0).all()
            col = cb2[t] * 8 + j // 16
            for rep in range(8):
                idx2[16 * rep + j % 16, col] = trow.astype(np.int16)

        in_maps.append({
            "xg": xg.astype(ml_dtypes.bfloat16),
            "sel1": sel1.reshape(P, C1 * P).astype(ml_dtypes.bfloat16),
            "idx1": idx1,
            "sel2": sel2.reshape(P, C2 * P),
            "idx2": idx2,
            "W1p": W1,
            "W2p": W2,
            "b1c": b1.reshape(D, 1).copy(),
            "b2c": b2.reshape(D, 1).copy(),
        })

    tables = dict(T1=T1, K1=K1, cb1=cb1, C1=C1, XROWS=XROWS,
                  T2=T2, K2=K2, cb2=cb2, C2=C2, TAB2R=TAB2R)
    meta = dict(gs=gs)
    return in_maps, tables, meta


def _build(tables):
    import concourse.bass as bass  # noqa: F401
    import concourse.tile as tile
    from concourse import mybir, bacc, library_config

    T1 = tables["T1"]
    K1 = tables["K1"]
    cb1 = tables["cb1"]
    C1 = tables["C1"]
    XROWS = tables["XROWS"]
    T2 = tables["T2"]
    K2 = tables["K2"]
    cb2 = tables["cb2"]
    C2 = tables["C2"]
    TAB2R = tables["TAB2R"]

    f32 = mybir.dt.float32
    bf16 = mybir.dt.bfloat16
    i16 = mybir.dt.int16
    AOP = mybir.AluOpType

    nc = bacc.Bacc("TRN2", target_bir_lowering=False, debug=False,
                   num_devices=NCORES, dynamic_dma_scratch_size=32768)

    xg = nc.declare_dram_parameter("xg", [XROWS, IN_DIM], bf16, isOutput=False)
    sel1 = nc.declare_dram_parameter("sel1", [P, C1 * P], bf16, isOutput=False)
    idx1 = nc.declare_dram_parameter("idx1", [P, C1 * 8], i16, isOutput=False)
    sel2 = nc.declare_dram_parameter("sel2", [P, C2 * P], f32, isOutput=False)
    idx2 = nc.declare_dram_parameter("idx2", [P, C2 * 8], i16, isOutput=False)
    W1p = nc.declare_dram_parameter("W1p", [IN_DIM, D], f32, isOutput=False)
    W2p = nc.declare_dram_parameter("W2p", [D, D], f32, isOutput=False)
    b1c = nc.declare_dram_parameter("b1c", [D, 1], f32, isOutput=False)
    b2c = nc.declare_dram_parameter("b2c", [D, 1], f32, isOutput=False)
    pool_out = nc.declare_dram_parameter("pool_out", [D, T2 * 64], f32,
                                         isOutput=True)

    tab2_mine = nc.dram_tensor("tab2_mine", [T1 * P, D], f32)
    tab2_full = nc.dram_tensor("tab2_full", [TAB2R, D], f32,
                               addr_space="Shared")
    groups = [list(range(NCORES))]

    from contextlib import ExitStack
    with ExitStack() as top:
        tc = top.enter_context(tile.TileContext(nc))
        nc.gpsimd.load_library(library_config.mlp)
        const = top.enter_context(tc.tile_pool(name="const", bufs=1))
        W1_t = const.tile([IN_DIM, D], f32)
        nc.sync.dma_start(out=W1_t[:], in_=W1p[:])
        W2_t = const.tile([D, D], f32)
        nc.sync.dma_start(out=W2_t[:], in_=W2p[:])
        b1_t = const.tile([D, 1], f32)
        nc.sync.dma_start(out=b1_t[:], in_=b1c[:])
        b2_t = const.tile([D, 1], f32)
        nc.sync.dma_start(out=b2_t[:], in_=b2c[:])

        # ---- layer 1: aggregate x, then @W1 (+b1), then @W2 -> tab2 ----
        with tc.tile_pool(name="l1", bufs=2) as l1p, \
             tc.tile_pool(name="l1f", bufs=2) as l1f, \
             tc.tile_pool(name="ps_a", bufs=2, space="PSUM") as psa, \
             tc.tile_pool(name="ps_b", bufs=2, space="PSUM") as psb:
            for t in range(T1):
                k = int(K1[t])
                cb = int(cb1[t])
                sel_t = l1p.tile([P, k * P], bf16, tag="sel")
                nc.scalar.dma_start(out=sel_t[:], in_=sel1[:, cb * P:(cb + k) * P])
                idx_t = l1p.tile([P, k * 8], i16, tag="idx")
                nc.scalar.dma_start(out=idx_t[:], in_=idx1[:, cb * 8:(cb + k) * 8])
                gbx = l1p.tile([P, k, IN_DIM], bf16, tag="gbx")
                ni = k * P
                nc.gpsimd.dma_gather(gbx[:], xg[:], idx_t[:], ni, ni, IN_DIM,
                                     single_packet=False)
                xaT = psa.tile([P, P], f32, tag="xaT")
                for kk in range(k):
                    nc.tensor.matmul(out=xaT[:], lhsT=gbx[:, kk, :],
                                     rhs=sel_t[:, kk * P:(kk + 1) * P],
                                     start=(kk == 0), stop=(kk == k - 1))
                xaS = l1f.tile([P, P], f32, tag="xaS")
                nc.vector.tensor_copy(out=xaS[:], in_=xaT[:])
                h1T = psb.tile([D, P], f32, tag="h1T")
                nc.tensor.matmul(out=h1T[:], lhsT=W1_t[:], rhs=xaS[:],
                                 start=True, stop=True)
                h1S = l1f.tile([D, P], f32, tag="h1S")
                nc.vector.tensor_scalar_add(out=h1S[:], in0=h1T[:],
                                            scalar1=b1_t[:])
                t2p = psb.tile([P, D], f32, tag="t2p")
                nc.tensor.matmul(out=t2p[:], lhsT=h1S[:], rhs=W2_t[:],
                                 start=True, stop=True)
                t2S = l1f.tile([P, D], f32, tag="t2S")
                nc.vector.tensor_copy(out=t2S[:], in_=t2p[:])
                nc.sync.dma_start(out=tab2_mine[t * P:(t + 1) * P, :],
                                  in_=t2S[:])

        nc.gpsimd.collective_compute(
            "AllGather", AOP.bypass, replica_groups=groups,
            ins=[tab2_mine[:]], outs=[tab2_full[:]])

        # ---- layer 2 + pooling head ----
        with tc.tile_pool(name="l2", bufs=2) as l2p, \
             tc.tile_pool(name="l2f", bufs=2) as l2f, \
             tc.tile_pool(name="ps_c", bufs=2, space="PSUM") as psc:
            for j in range(T2):
                k = int(K2[j])
                cb = int(cb2[j])
                sel_t = l2p.tile([P, k * P], f32, tag="sel2")
                nc.scalar.dma_start(out=sel_t[:], in_=sel2[:, cb * P:(cb + k) * P])
                idx_t = l2p.tile([P, k * 8], i16, tag="idx2")
                nc.scalar.dma_start(out=idx_t[:], in_=idx2[:, cb * 8:(cb + k) * 8])
                gb2 = l2p.tile([P, k, D], f32, tag="gb2")
                ni = k * P
                nc.gpsimd.dma_gather(gb2[:], tab2_full[:], idx_t[:], ni, ni, D,
                                     single_packet=False)
                h2T = psc.tile([D, P], f32, tag="h2T")
                for kk in range(k):
                    nc.tensor.matmul(out=h2T[:], lhsT=gb2[:, kk, :],
                                     rhs=sel_t[:, kk * P:(kk + 1) * P],
                                     start=(kk == 0), stop=(kk == k - 1))
                h2S = l2f.tile([D, P], f32, tag="h2S")
                nc.vector.tensor_copy(out=h2S[:], in_=h2T[:])
                pm = l2f.tile([D, 64], f32, tag="pm")
                nc.vector.tensor_tensor(out=pm[:], in0=h2S[:, 0:64],
                                        in1=h2S[:, 64:128], op=AOP.max)
                ot = l2f.tile([D, 64], f32, tag="ot")
                nc.vector.tensor_scalar_add(out=ot[:], in0=pm[:],
                                            scalar1=b2_t[:])
                nc.sync.dma_start(out=pool_out[:, j * 64:(j + 1) * 64],
                                  in_=ot[:])

    nc.compile()
    return nc


LAST_RESULTS = None


def kernel(**inputs):
    global LAST_RESULTS
    from concourse.bass_utils import run_bass_kernel_spmd

    in_maps, tables, meta = _prepare(inputs)
    nc = _build(tables)
    res = run_bass_kernel_spmd(nc, in_maps, list(range(NCORES)))
    LAST_RESULTS = res
    gs = meta["gs"]
    out = np.zeros((B, D), np.float32)
    for c in range(NCORES):
        po = np.asarray(res.results[c]["pool_out"], dtype=np.float32)
        for j in range((len(gs[c]) + 63) // 64):
            gsel = gs[c][j * 64:(j + 1) * 64]
            out[gsel] = po[:, j * 64:j * 64 + len(gsel)].T
    return out


# revision 11
# speedup vs baseline: 40.0391x; 1.6047x over previous
"""GCN (2x GCNConv + graclus-style max-pool head) on 8 Trainium2 NeuronCores.

v3 strategy — prune + linearity + host-built routing + streamed edge data:
  - The output reads h2 at only 512 nodes (first cluster pair per graph).
    Layer-2 aggregation is computed only for those dst nodes (~8.7K edges),
    and layer-1 aggregation only for the ~8.4K src nodes feeding them
    (~150K edges) — a ~20x cut vs the full 2x1.6M edge passes.
  - Linearity: sum_e norm_e * (x@W1)[src] == (sum_e norm_e * x[src]) @ W1.
  - Layer-1 edge features are pre-laid-out on host per edge slot
    (x[src_e] duplicated into chunk order, bf16), so the device just
    streams them with static HWDGE DMAs — no SWDGE gather descriptors.
    Only layer 2 needs a real dma_gather (its table is device-computed).
  - The GCN normalization dinv[src]*w*dinv[dst] is folded into host-built
    per-chunk selection matrices sel[e, lane] (one TensorE matmul per
    128-edge chunk accumulates the segment-sum in PSUM).
  - tab2 is AllGathered per tile (tile-major layout [T1, 8, 128, 64]) so
    the collectives overlap layer-1 compute of later tiles.
  - Layer 1 per tile:  xaggT = sum_k xe_k^T @ sel_k    [128 xdim, 128 nodes]
                       h1T   = W1^T @ xaggT + b1       [64, 128]
                       tab2  = h1T^T @ W2              [128, 64] -> AllGather
  - Layer 2 per pool tile (64 graphs: lanes 0-63 = even pair member,
    64-127 = odd):     h2T = sum_k gb2_k^T @ sel2_k    [64, 128]
                       out = max(h2T[:, :64], h2T[:, 64:]) + b2
"""

import sys

sys.path.insert(0, "/opt/trn_rl_repo")

import numpy as np
import ml_dtypes

N = 100000
E = 1600000
B = 256
IN_DIM = 128
OUT_DIM = 64
NCORES = 8
NS = N // NCORES
P = 128
D = OUT_DIM


def _prepare(inputs):
    x = np.asarray(inputs["x"], dtype=np.float32)
    edge_index = np.asarray(inputs["edge_index"]).astype(np.int64)
    ew = np.asarray(inputs["edge_weight"], dtype=np.float32)
    batch = np.asarray(inputs["batch"]).astype(np.int64)
    W1 = np.asarray(inputs["W1"], dtype=np.float32)
    b1 = np.asarray(inputs["b1"], dtype=np.float32)
    W2 = np.asarray(inputs["W2"], dtype=np.float32)
    b2 = np.asarray(inputs["b2"], dtype=np.float32)

    src0, dst0 = edge_index[0], edge_index[1]
    deg = np.zeros(N, np.float64)
    np.add.at(deg, dst0, ew.astype(np.float64))
    deg += 1.0
    dinv = (1.0 / np.sqrt(deg)).astype(np.float32)

    srcA = np.concatenate([src0, np.arange(N, dtype=np.int64)])
    dstA = np.concatenate([dst0, np.arange(N, dtype=np.int64)])
    wA = np.concatenate([ew, np.ones(N, np.float32)])
    norm = dinv[srcA] * wA * dinv[dstA]

    # CSR by dst
    order = np.argsort(dstA, kind="stable")
    srcS = srcA[order]
    normS = norm[order]
    dcnt = np.bincount(dstA, minlength=N)
    dstart = np.zeros(N + 1, np.int64)
    np.cumsum(dcnt, out=dstart[1:])

    # pooling head: first cluster of each graph
    ncl = N // 2
    bp = batch[0::2]
    first = np.full(B, np.iinfo(np.int64).max)
    np.minimum.at(first, bp, np.arange(ncl, dtype=np.int64))
    cl = np.clip(first, 0, ncl - 1)
    evens = 2 * cl
    odds = evens + 1
    gown = evens // NS

    gs = [np.nonzero(gown == c)[0] for c in range(NCORES)]
    T2 = max(1, max((len(g) + 63) // 64 for g in gs))

    # ---- layer-2 edge sets: per (core, pool tile): (src, lane, norm) ----
    e2 = [[None] * T2 for _ in range(NCORES)]
    for c in range(NCORES):
        for j in range(T2):
            gsel = gs[c][j * 64:(j + 1) * 64]
            ss, ll, nn = [], [], []
            for pos, g in enumerate(gsel):
                for parity, v in ((0, evens[g]), (1, odds[g])):
                    s0, s1 = dstart[v], dstart[v + 1]
                    ss.append(srcS[s0:s1])
                    nn.append(normS[s0:s1])
                    ll.append(np.full(s1 - s0, 64 * parity + pos, np.int64))
            if ss:
                e2[c][j] = (np.concatenate(ss), np.concatenate(ll),
                            np.concatenate(nn))
            else:
                e2[c][j] = (np.zeros(0, np.int64), np.zeros(0, np.int64),
                            np.zeros(0, np.float32))

    # ---- S2: distinct srcs over all layer-2 edges; packed by owner ----
    allsrc2 = np.concatenate([e2[c][j][0] for c in range(NCORES)
                              for j in range(T2)])
    S2 = np.unique(allsrc2)
    S2_c = [S2[S2 // NS == c] for c in range(NCORES)]
    T1 = max(1, max((len(s) + P - 1) // P for s in S2_c))
    # tile-major tab2 layout: row = t*(8*P) + c*P + lane
    tab2row = np.full(N, -1, np.int64)
    for c in range(NCORES):
        pos = np.arange(len(S2_c[c]))
        tab2row[S2_c[c]] = (pos // P) * (NCORES * P) + c * P + pos % P
    TAB2R = NCORES * T1 * P
    assert TAB2R <= 32767, TAB2R

    # ---- layer-1 edge sets: per (core, tile): (src, lane, norm) ----
    e1 = [[None] * T1 for _ in range(NCORES)]
    for c in range(NCORES):
        for t in range(T1):
            nodes = S2_c[c][t * P:(t + 1) * P]
            ss, ll, nn = [], [], []
            for lane, v in enumerate(nodes):
                s0, s1 = dstart[v], dstart[v + 1]
                ss.append(srcS[s0:s1])
                nn.append(normS[s0:s1])
                ll.append(np.full(s1 - s0, lane, np.int64))
            if ss:
                e1[c][t] = (np.concatenate(ss), np.concatenate(ll),
                            np.concatenate(nn))
            else:
                e1[c][t] = (np.zeros(0, np.int64), np.zeros(0, np.int64),
                            np.zeros(0, np.float32))

    # chunk capacities (uniform across cores for SPMD)
    K1 = [max(1, max((len(e1[c][t][0]) + P - 1) // P for c in range(NCORES)))
          for t in range(T1)]
    cb1 = np.concatenate([[0], np.cumsum(K1)]).astype(np.int64)
    C1 = int(cb1[-1])
    K2 = [max(1, max((len(e2[c][j][0]) + P - 1) // P for c in range(NCORES)))
          for j in range(T2)]
    cb2 = np.concatenate([[0], np.cumsum(K2)]).astype(np.int64)
    C2 = int(cb2[-1])

    x16 = x.astype(ml_dtypes.bfloat16)

    in_maps = []
    for c in range(NCORES):
        # per-edge-slot x rows (host-side gather), chunk-ordered
        xe = np.zeros((P, C1, IN_DIM), ml_dtypes.bfloat16)
        sel1 = np.zeros((P, C1, P), np.float32)
        for t in range(T1):
            ss, ll, nn = e1[c][t]
            j = np.arange(len(ss))
            xe[j % P, cb1[t] + j // P, :] = x16[ss]
            sel1[j % P, cb1[t] + j // P, ll] = nn

        sel2 = np.zeros((P, C2, P), np.float32)
        idx2 = np.zeros((P, C2 * 8), np.int16)
        for t in range(T2):
            ss, ll, nn = e2[c][t]
            j = np.arange(len(ss))
            sel2[j % P, cb2[t] + j // P, ll] = nn
            trow = tab2row[ss]
            assert (trow >= 0).all()
            col = cb2[t] * 8 + j // 16
            for rep in range(8):
                idx2[16 * rep + j % 16, col] = trow.astype(np.int16)

        in_maps.append({
            "xe": xe.reshape(P, C1 * IN_DIM),
            "sel1": sel1.reshape(P, C1 * P).astype(ml_dtypes.bfloat16),
            "sel2": sel2.reshape(P, C2 * P),
            "idx2": idx2,
            "W1p": W1,
            "W2p": W2,
            "b1c": b1.reshape(D, 1).copy(),
            "b2c": b2.reshape(D, 1).copy(),
        })

    tables = dict(T1=T1, K1=K1, cb1=cb1, C1=C1,
                  T2=T2, K2=K2, cb2=cb2, C2=C2, TAB2R=TAB2R)
    meta = dict(gs=gs)
    return in_maps, tables, meta


def _build(tables):
    import concourse.bass as bass  # noqa: F401
    import concourse.tile as tile
    from concourse import mybir, bacc, library_config

    T1 = tables["T1"]
    K1 = tables["K1"]
    cb1 = tables["cb1"]
    C1 = tables["C1"]
    T2 = tables["T2"]
    K2 = tables["K2"]
    cb2 = tables["cb2"]
    C2 = tables["C2"]
    TAB2R = tables["TAB2R"]

    f32 = mybir.dt.float32
    bf16 = mybir.dt.bfloat16
    i16 = mybir.dt.int16
    AOP = mybir.AluOpType

    nc = bacc.Bacc("TRN2", target_bir_lowering=False, debug=False,
                   num_devices=NCORES, dynamic_dma_scratch_size=32768)

    xe = nc.declare_dram_parameter("xe", [P, C1 * IN_DIM], bf16, isOutput=False)
    sel1 = nc.declare_dram_parameter("sel1", [P, C1 * P], bf16, isOutput=False)
    sel2 = nc.declare_dram_parameter("sel2", [P, C2 * P], f32, isOutput=False)
    idx2 = nc.declare_dram_parameter("idx2", [P, C2 * 8], i16, isOutput=False)
    W1p = nc.declare_dram_parameter("W1p", [IN_DIM, D], f32, isOutput=False)
    W2p = nc.declare_dram_parameter("W2p", [D, D], f32, isOutput=False)
    b1c = nc.declare_dram_parameter("b1c", [D, 1], f32, isOutput=False)
    b2c = nc.declare_dram_parameter("b2c", [D, 1], f32, isOutput=False)
    pool_out = nc.declare_dram_parameter("pool_out", [D, T2 * 64], f32,
                                         isOutput=True)

    tab2_mine = nc.dram_tensor("tab2_mine", [T1 * P, D], f32)
    tab2_full = nc.dram_tensor("tab2_full", [TAB2R, D], f32,
                               addr_space="Shared")
    groups = [list(range(NCORES))]

    from contextlib import ExitStack
    with ExitStack() as top:
        tc = top.enter_context(tile.TileContext(nc))
        nc.gpsimd.load_library(library_config.mlp)
        const = top.enter_context(tc.tile_pool(name="const", bufs=1))
        W1_t = const.tile([IN_DIM, D], f32)
        nc.sync.dma_start(out=W1_t[:], in_=W1p[:])
        W2_t = const.tile([D, D], f32)
        nc.sync.dma_start(out=W2_t[:], in_=W2p[:])
        b1_t = const.tile([D, 1], f32)
        nc.sync.dma_start(out=b1_t[:], in_=b1c[:])
        b2_t = const.tile([D, 1], f32)
        nc.sync.dma_start(out=b2_t[:], in_=b2c[:])
        # prefetch layer-2 routing while layer 1 runs
        sel2_t = const.tile([P, C2 * P], f32)
        nc.scalar.dma_start(out=sel2_t[:], in_=sel2[:])
        idx2_t = const.tile([P, C2 * 8], i16)
        nc.scalar.dma_start(out=idx2_t[:], in_=idx2[:])

        # ---- layer 1: aggregate x, then @W1 (+b1), then @W2 -> tab2 ----
        with tc.tile_pool(name="l1", bufs=3) as l1p, \
             tc.tile_pool(name="l1f", bufs=2) as l1f, \
             tc.tile_pool(name="ps_a", bufs=2, space="PSUM") as psa, \
             tc.tile_pool(name="ps_b", bufs=2, space="PSUM") as psb:
            for t in range(T1):
                k = int(K1[t])
                cb = int(cb1[t])
                sel_t = l1p.tile([P, k * P], bf16, tag="sel")
                nc.scalar.dma_start(out=sel_t[:],
                                    in_=sel1[:, cb * P:(cb + k) * P])
                xe_t = l1p.tile([P, k, IN_DIM], bf16, tag="xe")
                nc.sync.dma_start(out=xe_t[:],
                                  in_=xe[:, cb * IN_DIM:(cb + k) * IN_DIM])
                xaT = psa.tile([P, P], f32, tag="xaT")
                for kk in range(k):
                    nc.tensor.matmul(out=xaT[:], lhsT=xe_t[:, kk, :],
                                     rhs=sel_t[:, kk * P:(kk + 1) * P],
                                     start=(kk == 0), stop=(kk == k - 1))
                xaS = l1f.tile([P, P], f32, tag="xaS")
                nc.vector.tensor_copy(out=xaS[:], in_=xaT[:])
                h1T = psb.tile([D, P], f32, tag="h1T")
                nc.tensor.matmul(out=h1T[:], lhsT=W1_t[:], rhs=xaS[:],
                                 start=True, stop=True)
                h1S = l1f.tile([D, P], f32, tag="h1S")
                nc.vector.tensor_scalar_add(out=h1S[:], in0=h1T[:],
                                            scalar1=b1_t[:])
                t2p = psb.tile([P, D], f32, tag="t2p")
                nc.tensor.matmul(out=t2p[:], lhsT=h1S[:], rhs=W2_t[:],
                                 start=True, stop=True)
                t2S = l1f.tile([P, D], f32, tag="t2S")
                nc.vector.tensor_copy(out=t2S[:], in_=t2p[:])
                nc.sync.dma_start(out=tab2_mine[t * P:(t + 1) * P, :],
                                  in_=t2S[:])
                # per-tile AllGather overlaps later tiles' compute
                nc.gpsimd.collective_compute(
                    "AllGather", AOP.bypass, replica_groups=groups,
                    ins=[tab2_mine[t * P:(t + 1) * P, :]],
                    outs=[tab2_full[t * NCORES * P:(t + 1) * NCORES * P, :]])

        # ---- layer 2 + pooling head ----
        with tc.tile_pool(name="l2", bufs=2) as l2p, \
             tc.tile_pool(name="l2f", bufs=2) as l2f, \
             tc.tile_pool(name="ps_c", bufs=2, space="PSUM") as psc:
            for j in range(T2):
                k = int(K2[j])
                cb = int(cb2[j])
                gb2 = l2p.tile([P, k, D], f32, tag="gb2")
                ni = k * P
                nc.gpsimd.dma_gather(gb2[:], tab2_full[:],
                                     idx2_t[:, cb * 8:(cb + k) * 8],
                                     ni, ni, D, single_packet=False)
                h2T = psc.tile([D, P], f32, tag="h2T")
                for kk in range(k):
                    nc.tensor.matmul(out=h2T[:], lhsT=gb2[:, kk, :],
                                     rhs=sel2_t[:, (cb + kk) * P:(cb + kk + 1) * P],
                                     start=(kk == 0), stop=(kk == k - 1))
                h2S = l2f.tile([D, P], f32, tag="h2S")
                nc.vector.tensor_copy(out=h2S[:], in_=h2T[:])
                pm = l2f.tile([D, 64], f32, tag="pm")
                nc.vector.tensor_tensor(out=pm[:], in0=h2S[:, 0:64],
                                        in1=h2S[:, 64:128], op=AOP.max)
                ot = l2f.tile([D, 64], f32, tag="ot")
                nc.vector.tensor_scalar_add(out=ot[:], in0=pm[:],
                                            scalar1=b2_t[:])
                nc.sync.dma_start(out=pool_out[:, j * 64:(j + 1) * 64],
                                  in_=ot[:])

    nc.compile()
    return nc


LAST_RESULTS = None


def kernel(**inputs):
    global LAST_RESULTS
    from concourse.bass_utils import run_bass_kernel_spmd

    in_maps, tables, meta = _prepare(inputs)
    nc = _build(tables)
    res = run_bass_kernel_spmd(nc, in_maps, list(range(NCORES)))
    LAST_RESULTS = res
    gs = meta["gs"]
    out = np.zeros((B, D), np.float32)
    for c in range(NCORES):
        po = np.asarray(res.results[c]["pool_out"], dtype=np.float32)
        for j in range((len(gs[c]) + 63) // 64):
            gsel = gs[c][j * 64:(j + 1) * 64]
            out[gsel] = po[:, j * 64:j * 64 + len(gsel)].T
    return out


# revision 13
# speedup vs baseline: 113.6101x; 2.8375x over previous
"""GCN (2x GCNConv + graclus-style max-pool head) on 8 Trainium2 NeuronCores.

v4 strategy — prune + linearity + full per-core replication (no collectives):
  - The output reads h2 at only 512 nodes (first cluster pair per graph).
    Layer-2 aggregation is needed for only those dst nodes (~8.7K edges),
    and layer-1 aggregation only for the ~8.4K src nodes feeding them
    (~150K edges) — a ~20x cut vs the full 2x1.6M edge passes.
  - Graphs are partitioned across the 8 cores; each core REPLICATES the
    layer-1 work for exactly the src nodes its own layer-2 edges consume
    (~1060 nodes, ~18K edges per core — same volume as distributing by
    owner, but with zero cross-core communication: no AllGather, no
    gathers, no GpSimd at all).
  - Linearity: sum_e norm_e * (x@W1)[src] == (sum_e norm_e * x[src]) @ W1.
    Layer-1 edge features x[src_e] are pre-laid-out on host per edge slot
    (bf16, chunk order) and streamed with static HWDGE DMAs.
  - The GCN normalization dinv[src]*w*dinv[dst] is folded into host-built
    per-chunk selection matrices sel[e, lane] (one TensorE matmul per
    128-edge chunk accumulates the segment-sum in PSUM).
  - Layer 2 is a dense routing matmul: A2[r, lane] = sum of norm over
    edges (src-row r -> pooled lane), host-built per src tile, so h2
    accumulates in PSUM directly from the layer-1 SBUF tiles.
  - Per tile t:        xaggT = sum_k xe_k^T @ sel_k    [128 xdim, 128 nodes]
                       h1T   = W1^T @ xaggT + b1       [64, 128]
                       tab2  = h1T^T @ W2              [128, 64]  (SBUF)
                       h2T  += tab2^T @ A2_t           [64, 128]  (PSUM)
  - Pool tiles (64 graphs: lanes 0-63 = even pair member, 64-127 = odd):
                       out = max(h2T[:, :64], h2T[:, 64:]) + b2
"""

import sys

sys.path.insert(0, "/opt/trn_rl_repo")

import numpy as np
import ml_dtypes

N = 100000
E = 1600000
B = 256
IN_DIM = 128
OUT_DIM = 64
NCORES = 8
NS = N // NCORES
P = 128
D = OUT_DIM


def _prepare(inputs):
    x = np.asarray(inputs["x"], dtype=np.float32)
    edge_index = np.asarray(inputs["edge_index"]).astype(np.int64)
    ew = np.asarray(inputs["edge_weight"], dtype=np.float32)
    batch = np.asarray(inputs["batch"]).astype(np.int64)
    W1 = np.asarray(inputs["W1"], dtype=np.float32)
    b1 = np.asarray(inputs["b1"], dtype=np.float32)
    W2 = np.asarray(inputs["W2"], dtype=np.float32)
    b2 = np.asarray(inputs["b2"], dtype=np.float32)

    src0, dst0 = edge_index[0], edge_index[1]
    deg = np.zeros(N, np.float64)
    np.add.at(deg, dst0, ew.astype(np.float64))
    deg += 1.0
    dinv = (1.0 / np.sqrt(deg)).astype(np.float32)

    srcA = np.concatenate([src0, np.arange(N, dtype=np.int64)])
    dstA = np.concatenate([dst0, np.arange(N, dtype=np.int64)])
    wA = np.concatenate([ew, np.ones(N, np.float32)])
    norm = dinv[srcA] * wA * dinv[dstA]

    # CSR by dst
    order = np.argsort(dstA, kind="stable")
    srcS = srcA[order]
    normS = norm[order]
    dcnt = np.bincount(dstA, minlength=N)
    dstart = np.zeros(N + 1, np.int64)
    np.cumsum(dcnt, out=dstart[1:])

    # pooling head: first cluster of each graph
    ncl = N // 2
    bp = batch[0::2]
    first = np.full(B, np.iinfo(np.int64).max)
    np.minimum.at(first, bp, np.arange(ncl, dtype=np.int64))
    cl = np.clip(first, 0, ncl - 1)
    evens = 2 * cl
    odds = evens + 1

    # graphs partitioned across cores (balanced round-robin by id)
    gs = [np.arange(B)[c::NCORES] for c in range(NCORES)]
    T2 = max(1, max((len(g) + 63) // 64 for g in gs))

    # ---- layer-2 edge sets: per (core, pool tile): (src, lane, norm) ----
    e2 = [[None] * T2 for _ in range(NCORES)]
    for c in range(NCORES):
        for j in range(T2):
            gsel = gs[c][j * 64:(j + 1) * 64]
            ss, ll, nn = [], [], []
            for pos, g in enumerate(gsel):
                for parity, v in ((0, evens[g]), (1, odds[g])):
                    s0, s1 = dstart[v], dstart[v + 1]
                    ss.append(srcS[s0:s1])
                    nn.append(normS[s0:s1])
                    ll.append(np.full(s1 - s0, 64 * parity + pos, np.int64))
            if ss:
                e2[c][j] = (np.concatenate(ss), np.concatenate(ll),
                            np.concatenate(nn))
            else:
                e2[c][j] = (np.zeros(0, np.int64), np.zeros(0, np.int64),
                            np.zeros(0, np.float32))

    # ---- per-core local src sets (replicated layer-1 work) ----
    S2loc = [np.unique(np.concatenate([e2[c][j][0] for j in range(T2)]))
             for c in range(NCORES)]
    T1 = max(1, max((len(s) + P - 1) // P for s in S2loc))

    # ---- layer-1 edge sets: per (core, tile): (src, lane, norm) ----
    e1 = [[None] * T1 for _ in range(NCORES)]
    for c in range(NCORES):
        for t in range(T1):
            nodes = S2loc[c][t * P:(t + 1) * P]
            ss, ll, nn = [], [], []
            for lane, v in enumerate(nodes):
                s0, s1 = dstart[v], dstart[v + 1]
                ss.append(srcS[s0:s1])
                nn.append(normS[s0:s1])
                ll.append(np.full(s1 - s0, lane, np.int64))
            if ss:
                e1[c][t] = (np.concatenate(ss), np.concatenate(ll),
                            np.concatenate(nn))
            else:
                e1[c][t] = (np.zeros(0, np.int64), np.zeros(0, np.int64),
                            np.zeros(0, np.float32))

    # chunk capacities (uniform across cores for SPMD)
    K1 = [max(1, max((len(e1[c][t][0]) + P - 1) // P for c in range(NCORES)))
          for t in range(T1)]
    cb1 = np.concatenate([[0], np.cumsum(K1)]).astype(np.int64)
    C1 = int(cb1[-1])

    x16 = x.astype(ml_dtypes.bfloat16)

    in_maps = []
    for c in range(NCORES):
        # local row index for this core's src set
        loc = S2loc[c]

        # per-edge-slot x rows (host-side gather), chunk-ordered
        xe = np.zeros((P, C1, IN_DIM), ml_dtypes.bfloat16)
        sel1 = np.zeros((P, C1, P), np.float32)
        for t in range(T1):
            ss, ll, nn = e1[c][t]
            j = np.arange(len(ss))
            xe[j % P, cb1[t] + j // P, :] = x16[ss]
            sel1[j % P, cb1[t] + j // P, ll] = nn

        # dense layer-2 routing: A2[r_lane, t, j, lane] summed over edges
        A2 = np.zeros((P, T1, T2, P), np.float32)
        for j in range(T2):
            ss, ll, nn = e2[c][j]
            pos = np.searchsorted(loc, ss)
            assert (loc[pos] == ss).all()
            np.add.at(A2, (pos % P, pos // P, j, ll), nn)

        in_maps.append({
            "xe": xe.reshape(P, C1 * IN_DIM),
            "sel1": sel1.reshape(P, C1 * P).astype(ml_dtypes.bfloat16),
            "A2": A2.reshape(P, T1 * T2 * P),
            "W1p": W1,
            "W2p": W2,
            "b1c": b1.reshape(D, 1).copy(),
            "b2c": b2.reshape(D, 1).copy(),
        })

    tables = dict(T1=T1, K1=K1, cb1=cb1, C1=C1, T2=T2)
    meta = dict(gs=gs)
    return in_maps, tables, meta


def _build(tables):
    import concourse.bass as bass  # noqa: F401
    import concourse.tile as tile
    from concourse import mybir, bacc

    T1 = tables["T1"]
    K1 = tables["K1"]
    cb1 = tables["cb1"]
    C1 = tables["C1"]
    T2 = tables["T2"]

    f32 = mybir.dt.float32
    bf16 = mybir.dt.bfloat16
    AOP = mybir.AluOpType

    nc = bacc.Bacc("TRN2", target_bir_lowering=False, debug=False,
                   num_devices=NCORES)

    xe = nc.declare_dram_parameter("xe", [P, C1 * IN_DIM], bf16, isOutput=False)
    sel1 = nc.declare_dram_parameter("sel1", [P, C1 * P], bf16, isOutput=False)
    A2 = nc.declare_dram_parameter("A2", [P, T1 * T2 * P], f32, isOutput=False)
    W1p = nc.declare_dram_parameter("W1p", [IN_DIM, D], f32, isOutput=False)
    W2p = nc.declare_dram_parameter("W2p", [D, D], f32, isOutput=False)
    b1c = nc.declare_dram_parameter("b1c", [D, 1], f32, isOutput=False)
    b2c = nc.declare_dram_parameter("b2c", [D, 1], f32, isOutput=False)
    pool_out = nc.declare_dram_parameter("pool_out", [D, T2 * 64], f32,
                                         isOutput=True)

    from contextlib import ExitStack
    with ExitStack() as top:
        tc = top.enter_context(tile.TileContext(nc))
        const = top.enter_context(tc.tile_pool(name="const", bufs=1))
        W1_t = const.tile([IN_DIM, D], f32)
        nc.sync.dma_start(out=W1_t[:], in_=W1p[:])
        W2_t = const.tile([D, D], f32)
        nc.sync.dma_start(out=W2_t[:], in_=W2p[:])
        b1_t = const.tile([D, 1], f32)
        nc.sync.dma_start(out=b1_t[:], in_=b1c[:])
        b2_t = const.tile([D, 1], f32)
        nc.sync.dma_start(out=b2_t[:], in_=b2c[:])
        A2_t = const.tile([P, T1 * T2 * P], f32)
        nc.scalar.dma_start(out=A2_t[:], in_=A2[:])

        with tc.tile_pool(name="l1", bufs=3) as l1p, \
             tc.tile_pool(name="l1f", bufs=2) as l1f, \
             tc.tile_pool(name="ps_a", bufs=2, space="PSUM") as psa, \
             tc.tile_pool(name="ps_b", bufs=2, space="PSUM") as psb, \
             tc.tile_pool(name="ps_h2", bufs=1, space="PSUM") as psh2, \
             tc.tile_pool(name="fin", bufs=1) as finp:
            h2T = [psh2.tile([D, P], f32, tag=f"h2T{j}", name=f"h2T{j}")
                   for j in range(T2)]
            for t in range(T1):
                k = int(K1[t])
                cb = int(cb1[t])
                sel_t = l1p.tile([P, k * P], bf16, tag="sel")
                nc.scalar.dma_start(out=sel_t[:],
                                    in_=sel1[:, cb * P:(cb + k) * P])
                xe_t = l1p.tile([P, k, IN_DIM], bf16, tag="xe")
                nc.sync.dma_start(out=xe_t[:],
                                  in_=xe[:, cb * IN_DIM:(cb + k) * IN_DIM])
                xaT = psa.tile([P, P], f32, tag="xaT")
                for kk in range(k):
                    nc.tensor.matmul(out=xaT[:], lhsT=xe_t[:, kk, :],
                                     rhs=sel_t[:, kk * P:(kk + 1) * P],
                                     start=(kk == 0), stop=(kk == k - 1))
                xaS = l1f.tile([P, P], f32, tag="xaS")
                nc.vector.tensor_copy(out=xaS[:], in_=xaT[:])
                h1T = psb.tile([D, P], f32, tag="h1T")
                nc.tensor.matmul(out=h1T[:], lhsT=W1_t[:], rhs=xaS[:],
                                 start=True, stop=True)
                h1S = l1f.tile([D, P], f32, tag="h1S")
                nc.vector.tensor_scalar_add(out=h1S[:], in0=h1T[:],
                                            scalar1=b1_t[:])
                t2p = psb.tile([P, D], f32, tag="t2p")
                nc.tensor.matmul(out=t2p[:], lhsT=h1S[:], rhs=W2_t[:],
                                 start=True, stop=True)
                t2S = l1f.tile([P, D], f32, tag="t2S")
                nc.vector.tensor_copy(out=t2S[:], in_=t2p[:])
                for j in range(T2):
                    nc.tensor.matmul(
                        out=h2T[j][:], lhsT=t2S[:],
                        rhs=A2_t[:, (t * T2 + j) * P:(t * T2 + j + 1) * P],
                        start=(t == 0), stop=(t == T1 - 1))

            for j in range(T2):
                h2S = finp.tile([D, P], f32, tag=f"h2S{j}")
                nc.vector.tensor_copy(out=h2S[:], in_=h2T[j][:])
                pm = finp.tile([D, 64], f32, tag=f"pm{j}")
                nc.vector.tensor_tensor(out=pm[:], in0=h2S[:, 0:64],
                                        in1=h2S[:, 64:128], op=AOP.max)
                ot = finp.tile([D, 64], f32, tag=f"ot{j}")
                nc.vector.tensor_scalar_add(out=ot[:], in0=pm[:],
                                            scalar1=b2_t[:])
                nc.sync.dma_start(out=pool_out[:, j * 64:(j + 1) * 64],
                                  in_=ot[:])

    nc.compile()
    return nc


LAST_RESULTS = None


def kernel(**inputs):
    global LAST_RESULTS
    from concourse.bass_utils import run_bass_kernel_spmd

    in_maps, tables, meta = _prepare(inputs)
    nc = _build(tables)
    res = run_bass_kernel_spmd(nc, in_maps, list(range(NCORES)))
    LAST_RESULTS = res
    gs = meta["gs"]
    out = np.zeros((B, D), np.float32)
    for c in range(NCORES):
        po = np.asarray(res.results[c]["pool_out"], dtype=np.float32)
        for j in range((len(gs[c]) + 63) // 64):
            gsel = gs[c][j * 64:(j + 1) * 64]
            out[gsel] = po[:, j * 64:j * 64 + len(gsel)].T
    return out
